# revision 11
# baseline (speedup 1.0000x reference)
"""GCN 3-layer kernel for Trainium2, 8-core SPMD — v2 (pipelined).

Math (per layer, PyG GCN convention, factorized):
    deg[d]  = indegree(d) + 1;  dinv = deg^-1/2
    y       = dinv[:,None] * (h @ W)                    (message table)
    agg[d]  = sum_{e: dst[e]=d} y[src[e]]               (edges only)
    h_next  = dinv[:,None] * agg + (dinv*y + b)         (self-loop + bias
              folded into a precomputed  y2b = dinv^2*(h@W) + b  table)

Distribution: destination-sharded across 8 cores (6272 nodes/core, padded
50176 total).  Per layer: every core's y rows are AllGathered into a full
bf16 table in DRAM; each core then gathers message rows for its incoming
edges with prepare_only dma_gather + trigger_dma (so the Q7 engine is held
only for descriptor generation and DMA drains overlap compute), and
scatter-adds them with one-hot matmuls on the PE (PSUM accumulation per
128-dst window).  The one-hot S for a whole window is built with a single
broadcast tensor_tensor is_equal per stream.  The next layer's y/y2b rows
are produced in the same per-window epilogue (transpose + matmul fused),
so each layer ends directly in its AllGather.

dma_gather indices are int16, so the y table is addressed via two base
offsets (row 0 for src < 25088, row 17408 for src >= 25088; 50176-17408 =
32768 rows exactly covers the int16 range).
"""

import numpy as np
import ml_dtypes

N_NODES = 50000
N_CORES = 8
PER_CORE = 6272            # 49 * 128
N_PAD = PER_CORE * N_CORES # 50176
N_WIN = PER_CORE // 128    # 49
HI_BASE = 17408            # hi gather base row; 50176-17408 = 32768
LO_HI_SPLIT = 25088        # src < split -> lo stream, else hi
F = 128                    # feature width (layer3 padded 64->128)
F_OUT = 64
GROUP_WINDOWS = 5          # windows per gather chunk
PREP_GATHER = True         # prepare_only + trigger_dma pipelined gathers

BF16 = ml_dtypes.bfloat16


def _wrap_idx16(idx: np.ndarray) -> np.ndarray:
    """Wrap a flat int16 index stream into the [128, n/16] layout dma_gather
    expects (element i at [i%16, i//16], replicated across the 8 groups of
    16 partitions)."""
    n = len(idx)
    assert n % 128 == 0
    cols = n // 16
    out = np.empty((128, cols), np.int16)
    w = idx.reshape(cols, 16).T  # [16, cols]
    for g in range(8):
        out[g * 16:(g + 1) * 16, :] = w
    return out


def _preprocess(edge_index: np.ndarray):
    """Host-side graph prep: degree norm, dst-sharding, per-window edge
    streams (lo/hi by source row), block padding shared across cores.
    Self-loops are NOT materialized as edges (handled via the y2b table)."""
    src = edge_index[0].astype(np.int64)
    dst = edge_index[1].astype(np.int64)
    deg = np.bincount(dst, minlength=N_NODES).astype(np.float64) + 1.0
    dinv = (1.0 / np.sqrt(deg)).astype(np.float32)
    dinv_pad = np.ones(N_PAD, np.float32)
    dinv_pad[:N_NODES] = dinv

    core_of = dst // PER_CORE
    win_of = (dst % PER_CORE) // 128
    dloc_of = dst % 128
    is_lo = src < LO_HI_SPLIT

    order = np.lexsort((dst, win_of, core_of))
    src_s, core_s, win_s, dloc_s, lo_s = (
        src[order], core_of[order], win_of[order], dloc_of[order], is_lo[order])

    # per (core, window, stream) counts
    counts = np.zeros((N_CORES, N_WIN, 2), np.int64)
    np.add.at(counts, (core_s, win_s, (~lo_s).astype(np.int64)), 1)
    # shared block counts per window (max over cores), at least 1 block each
    blk_lo = np.maximum(1, -(-counts[:, :, 0].max(axis=0) // 128))  # [N_WIN]
    blk_hi = np.maximum(1, -(-counts[:, :, 1].max(axis=0) // 128))  # [N_WIN]

    off_lo = np.concatenate([[0], np.cumsum(blk_lo * 128)])
    off_hi = np.concatenate([[0], np.cumsum(blk_hi * 128)])
    n_lo, n_hi = int(off_lo[-1]), int(off_hi[-1])

    idx_lo = np.zeros((N_CORES, n_lo), np.int16)
    idx_hi = np.zeros((N_CORES, n_hi), np.int16)
    dl_lo = np.full((N_CORES, n_lo), 999.0, np.float32)
    dl_hi = np.full((N_CORES, n_hi), 999.0, np.float32)

    keys = core_s * N_WIN + win_s
    bounds = np.searchsorted(keys, np.arange(N_CORES * N_WIN + 1))
    for c in range(N_CORES):
        for w in range(N_WIN):
            k = c * N_WIN + w
            sl = slice(bounds[k], bounds[k + 1])
            s_src = src_s[sl]; s_dl = dloc_s[sl]; s_lo = lo_s[sl]
            lo_src = s_src[s_lo]; lo_dl = s_dl[s_lo]
            hi_src = s_src[~s_lo]; hi_dl = s_dl[~s_lo]
            o = off_lo[w]
            idx_lo[c, o:o + len(lo_src)] = lo_src.astype(np.int16)
            dl_lo[c, o:o + len(lo_src)] = lo_dl
            o = off_hi[w]
            idx_hi[c, o:o + len(hi_src)] = (hi_src - HI_BASE).astype(np.int16)
            dl_hi[c, o:o + len(hi_src)] = hi_dl

    return dinv_pad, blk_lo, blk_hi, off_lo, off_hi, idx_lo, idx_hi, dl_lo, dl_hi


def _build_and_run(inputs_np, dinv_pad, blk_lo, blk_hi, off_lo, off_hi,
                   idx_lo, idx_hi, dl_lo, dl_hi, trace=False, sim=False):
    import concourse.bacc as bacc
    import concourse.mybir as mybir
    from concourse.tile import TileContext
    from concourse import bass, bass_utils, library_config
    from concourse.masks import make_identity

    x = inputs_np["x"]
    Ws = [np.asarray(inputs_np[k], np.float32) for k in ("W1", "W2", "W3")]
    bs = [np.asarray(inputs_np[k], np.float32) for k in ("b1", "b2", "b3")]
    # pad W3/b3 to 128 output features
    W3p = np.zeros((F, F), np.float32); W3p[:, :F_OUT] = Ws[2]
    b3p = np.zeros(F, np.float32); b3p[:F_OUT] = bs[2]
    Ws[2], bs[2] = W3p, b3p

    n_lo, n_hi = idx_lo.shape[1], idx_hi.shape[1]
    G = GROUP_WINDOWS
    groups = [list(range(g, min(g + G, N_WIN))) for g in range(0, N_WIN, G)]
    glo = [(int(off_lo[g[0]]), int(off_lo[g[-1] + 1])) for g in groups]
    ghi = [(int(off_hi[g[0]]), int(off_hi[g[-1] + 1])) for g in groups]
    cap_lo = max(b - a for a, b in glo) // 128
    cap_hi = max(b - a for a, b in ghi) // 128

    nc = bacc.Bacc("TRN2", target_bir_lowering=False, debug=False,
                   num_devices=N_CORES, num_swdge_queues=2)
    dt = mybir.dt
    Alu = mybir.AluOpType
    Act = mybir.ActivationFunctionType

    # ---- kernel I/O -----------------------------------------------------
    t_xT = nc.dram_tensor("xT_own", [128, PER_CORE], dt.float32, kind="ExternalInput")
    t_W = [nc.dram_tensor(f"W{i+1}m", [F, F], dt.float32, kind="ExternalInput") for i in range(3)]
    t_b = [nc.dram_tensor(f"b{i+1}m", [128, F], dt.float32, kind="ExternalInput") for i in range(3)]
    t_dinv = nc.dram_tensor("dinv_own", [128, N_WIN], dt.float32, kind="ExternalInput")
    t_dinv2 = nc.dram_tensor("dinv2_own", [128, N_WIN], dt.float32, kind="ExternalInput")
    t_iota = nc.dram_tensor("iota", [128, 128], dt.bfloat16, kind="ExternalInput")
    t_ilo = nc.dram_tensor("idx_lo", [128, n_lo // 16], dt.int16, kind="ExternalInput")
    t_ihi = nc.dram_tensor("idx_hi", [128, n_hi // 16], dt.int16, kind="ExternalInput")
    t_dlo = nc.dram_tensor("dl_lo", [128, n_lo // 128], dt.bfloat16, kind="ExternalInput")
    t_dhi = nc.dram_tensor("dl_hi", [128, n_hi // 128], dt.bfloat16, kind="ExternalInput")
    t_out = nc.dram_tensor("h_out", [PER_CORE, F_OUT], dt.float32, kind="ExternalOutput")

    with TileContext(nc) as tc:
        nc.gpsimd.load_library(library_config.mlp)
        gsem = [nc.alloc_semaphore(f"gsem{q}") for q in range(2)]
        mdone = nc.alloc_semaphore("mdone")
        with tc.tile_pool(name="const", bufs=1) as cpool, \
             tc.tile_pool(name="state", bufs=1) as spool, \
             tc.tile_pool(name="gath", bufs=2) as gpool, \
             tc.tile_pool(name="sbld", bufs=3) as sbld, \
             tc.tile_pool(name="work", bufs=3) as wpool, \
             tc.tile_pool(name="stg", bufs=2) as stg, \
             tc.tile_pool(name="psA", bufs=4, space="PSUM") as psA, \
             tc.tile_pool(name="psT", bufs=2, space="PSUM") as psT, \
             tc.tile_pool(name="ps2", bufs=2, space="PSUM") as ps2p, \
             tc.tile_pool(name="dram", bufs=1, space="DRAM") as dpool:

            # ---- constants ----
            c_W = [cpool.tile([F, F], dt.float32, tag=f"W{i}", name=f"cW{i}") for i in range(3)]
            c_b = [cpool.tile([128, F], dt.float32, tag=f"b{i}", name=f"cb{i}") for i in range(3)]
            c_dinv = cpool.tile([128, N_WIN], dt.float32, tag="dinv", name="dinv")
            c_dinv2 = cpool.tile([128, N_WIN], dt.float32, tag="dinv2", name="dinv2")
            c_iota = cpool.tile([128, 128], dt.bfloat16, tag="iota", name="iota")
            c_ilo = cpool.tile([128, n_lo // 16], dt.int16, tag="ilo", name="ilo")
            c_ihi = cpool.tile([128, n_hi // 16], dt.int16, tag="ihi", name="ihi")
            c_dlo = cpool.tile([128, n_lo // 128], dt.bfloat16, tag="dlo", name="dlo")
            c_dhi = cpool.tile([128, n_hi // 128], dt.bfloat16, tag="dhi", name="dhi")
            c_ident = cpool.tile([128, 128], dt.float32, tag="ident", name="ident")
            for i in range(3):
                nc.sync.dma_start(c_W[i][:], t_W[i][:])
                nc.sync.dma_start(c_b[i][:], t_b[i][:])
            nc.sync.dma_start(c_dinv[:], t_dinv[:])
            nc.sync.dma_start(c_dinv2[:], t_dinv2[:])
            nc.sync.dma_start(c_iota[:], t_iota[:])
            nc.sync.dma_start(c_ilo[:], t_ilo[:])
            nc.sync.dma_start(c_ihi[:], t_ihi[:])
            nc.sync.dma_start(c_dlo[:], t_dlo[:])
            nc.sync.dma_start(c_dhi[:], t_dhi[:])
            make_identity(nc, c_ident[:])

            # ---- persistent state: y2b = dinv^2*(h@W) + b, two generations
            y2b = [spool.tile([128, N_WIN, F], dt.float32, tag="y2b_a", name="y2b_a"),
                   spool.tile([128, N_WIN, F], dt.float32, tag="y2b_b", name="y2b_b")]

            y_fulls = [dpool.tile([N_PAD, F], dt.bfloat16, addr_space="Shared",
                                  name=f"y_full{i}") for i in range(3)]
            ag_ins = [dpool.tile([PER_CORE, F], dt.bfloat16, name=f"ag_in{i}")
                      for i in range(3)]

            def stage_y(ps, w, wi, layer_next, yst):
                """From PSUM ps = (h @ W_next) for window w: stage bf16 y row
                block and write fp32 y2b (self+bias) for the next layer."""
                nc.scalar.mul(yst[:, wi, :], ps[:], c_dinv[:, w:w + 1])
                nc.vector.scalar_tensor_tensor(
                    out=y2b[layer_next % 2][:, w, :], in0=ps[:],
                    scalar=c_dinv2[:, w:w + 1], in1=c_b[layer_next][:],
                    op0=Alu.mult, op1=Alu.add)

            def flush_y(g, layer_next, yst):
                """DMA the staged bf16 y rows of group g to the AG input."""
                w0, w1 = g[0], g[-1] + 1
                nc.sync.dma_start(
                    ag_ins[layer_next][w0 * 128:w1 * 128, :]
                    .rearrange("(t p) f -> p t f", p=128),
                    yst[:, :w1 - w0, :])

            # ---- layer 0 phase A: y1 = dinv*(x@W1) ----
            with tc.tile_pool(name="xp", bufs=1) as xpool:
                xT = xpool.tile([128, PER_CORE], dt.float32, tag="xT", name="xT")
                nc.sync.dma_start(xT[:], t_xT[:])
                for g in groups:
                    yst = stg.tile([128, G, F], dt.bfloat16, tag="yst", name="yst")
                    for wi, w in enumerate(g):
                        ps = psA.tile([128, F], dt.float32, tag="psA", space="PSUM")
                        nc.tensor.matmul(ps[:], lhsT=xT[:, w * 128:(w + 1) * 128],
                                         rhs=c_W[0][:], start=True, stop=True)
                        stage_y(ps, w, wi, 0, yst)
                    flush_y(g, 0, yst)
            nc.gpsimd.collective_compute(
                "AllGather", Alu.bypass,
                replica_groups=[list(range(N_CORES))],
                ins=[ag_ins[0].opt()], outs=[y_fulls[0].opt()])

            # ---- layers ----
            for layer in range(3):
                y_full = y_fulls[layer]
                for gi, g in enumerate(groups):
                    lo_a, lo_b = glo[gi]
                    hi_a, hi_b = ghi[gi]
                    nlo = lo_b - lo_a
                    nhi = hi_b - hi_a
                    m_lo = gpool.tile([128, cap_lo, F], dt.bfloat16, tag="mlo", name="mlo")
                    m_hi = gpool.tile([128, cap_hi, F], dt.bfloat16, tag="mhi", name="mhi")
                    kg = layer * len(groups) + gi  # global group ordinal
                    pk = dict(prepare_only=True) if PREP_GATHER else {}
                    if PREP_GATHER and kg >= 2:
                        # don't overwrite an m buffer still being read
                        # (bufs=2): group kg-2 must be fully consumed
                        nc.gpsimd.wait_ge(mdone, kg - 1)
                    nc.gpsimd.dma_gather(
                        out_ap=m_lo[:, :nlo // 128, :], in_ap=y_full[:],
                        idxs_ap=c_ilo[:, lo_a // 16:lo_b // 16],
                        num_idxs=nlo, num_idxs_reg=nlo, elem_size=F,
                        queue_num=0, single_packet=False,
                        sem=gsem[0] if PREP_GATHER else None, **pk)
                    if PREP_GATHER:
                        if kg >= 1:
                            # serialize same-queue drains so the shared sem's
                            # +16 batches can't interleave across gathers
                            nc.gpsimd.wait_ge(gsem[0], 16 * kg)
                        nc.gpsimd.trigger_dma(count=None, queue_num=0)
                    nc.gpsimd.dma_gather(
                        out_ap=m_hi[:, :nhi // 128, :], in_ap=y_full[HI_BASE:, :],
                        idxs_ap=c_ihi[:, hi_a // 16:hi_b // 16],
                        num_idxs=nhi, num_idxs_reg=nhi, elem_size=F,
                        queue_num=1, single_packet=False,
                        sem=gsem[1] if PREP_GATHER else None, **pk)
                    if PREP_GATHER:
                        if kg >= 1:
                            nc.gpsimd.wait_ge(gsem[1], 16 * kg)
                        nc.gpsimd.trigger_dma(count=None, queue_num=1)
                    yst = stg.tile([128, G, F], dt.bfloat16, tag="yst", name="yst")
                    ost = stg.tile([128, G, F_OUT], dt.float32, tag="ost", name="ost")
                    for wi, w in enumerate(g):
                        bl = int(blk_lo[w]); bh = int(blk_hi[w])
                        nblk = bl + bh
                        # one-hot S for the whole window, one op per stream
                        S = sbld.tile([128, bl + bh, 128], dt.bfloat16, tag="S", name="S")
                        B0l = int(off_lo[w]) // 128
                        B0h = int(off_hi[w]) // 128
                        dlo_b = (c_dlo[:, B0l:B0l + bl].unsqueeze(2)
                                 .broadcast_to([128, bl, 128]))
                        nc.vector.tensor_tensor(
                            out=S[:, :bl, :], in0=dlo_b,
                            in1=c_iota[:].unsqueeze(1).broadcast_to([128, bl, 128]),
                            op=Alu.is_equal)
                        dhi_b = (c_dhi[:, B0h:B0h + bh].unsqueeze(2)
                                 .broadcast_to([128, bh, 128]))
                        nc.vector.tensor_tensor(
                            out=S[:, bl:, :], in0=dhi_b,
                            in1=c_iota[:].unsqueeze(1).broadcast_to([128, bh, 128]),
                            op=Alu.is_equal)
                        # scatter-add via PSUM-accumulated one-hot matmuls
                        agg = psA.tile([128, F], dt.float32, tag="psA", space="PSUM")
                        if PREP_GATHER and wi == 0:
                            nc.tensor.wait_ge(gsem[0], 16 * (kg + 1))
                        k = 0
                        for b in range(bl):
                            nc.tensor.matmul(
                                agg[:], lhsT=S[:, k, :],
                                rhs=m_lo[:, B0l - lo_a // 128 + b, :],
                                start=(k == 0), stop=(k == nblk - 1))
                            k += 1
                        if PREP_GATHER and wi == 0:
                            nc.tensor.wait_ge(gsem[1], 16 * (kg + 1))
                        for b in range(bh):
                            nc.tensor.matmul(
                                agg[:], lhsT=S[:, k, :],
                                rhs=m_hi[:, B0h - hi_a // 128 + b, :],
                                start=(k == 0), stop=(k == nblk - 1))
                            k += 1
                        if PREP_GATHER and wi == len(g) - 1:
                            nc.tensor.sem_inc(mdone, 1)
                        # ---- epilogue: h = dinv*agg + y2b ----
                        h = wpool.tile([128, F], dt.float32, tag="h", name="h")
                        nc.vector.scalar_tensor_tensor(
                            out=h[:], in0=agg[:], scalar=c_dinv[:, w:w + 1],
                            in1=y2b[layer % 2][:, w, :], op0=Alu.mult, op1=Alu.add)
                        if layer < 2:
                            tp = psT.tile([128, 128], dt.float32, tag="tp", space="PSUM")
                            nc.tensor.transpose(tp[:], h[:], c_ident[:])
                            hT = wpool.tile([128, F], dt.float32, tag="hT", name="hT")
                            nc.scalar.copy(hT[:], tp[:])
                            ps2 = ps2p.tile([128, F], dt.float32, tag="ps2", space="PSUM")
                            nc.tensor.matmul(ps2[:], lhsT=hT[:], rhs=c_W[layer + 1][:],
                                             start=True, stop=True)
                            stage_y(ps2, w, wi, layer + 1, yst)
                        else:
                            nc.scalar.activation(ost[:, wi, :], h[:, :F_OUT], Act.Relu)
                    if layer < 2:
                        flush_y(g, layer + 1, yst)
                    else:
                        w0, w1 = g[0], g[-1] + 1
                        nc.sync.dma_start(
                            t_out[w0 * 128:w1 * 128, :]
                            .rearrange("(t p) f -> p t f", p=128),
                            ost[:, :w1 - w0, :])
                if layer < 2:
                    nc.gpsimd.collective_compute(
                        "AllGather", Alu.bypass,
                        replica_groups=[list(range(N_CORES))],
                        ins=[ag_ins[layer + 1].opt()],
                        outs=[y_fulls[layer + 1].opt()])

    nc.compile()

    # ---- per-core inputs ----
    xT_all = np.zeros((128, N_PAD), np.float32)
    xT_all[:, :N_NODES] = np.asarray(x, np.float32).T
    iota_m = np.broadcast_to(np.arange(128, dtype=np.float32), (128, 128)).astype(BF16)
    in_maps = []
    for c in range(N_CORES):
        rows = slice(c * PER_CORE, (c + 1) * PER_CORE)
        din = dinv_pad[rows].reshape(N_WIN, 128).T.copy()  # [128, N_WIN]
        in_map = {
            "xT_own": np.ascontiguousarray(xT_all[:, rows]),
            "dinv_own": din,
            "dinv2_own": din * din,
            "iota": iota_m.copy(),
            "idx_lo": _wrap_idx16(idx_lo[c]),
            "idx_hi": _wrap_idx16(idx_hi[c]),
            "dl_lo": dl_lo[c].reshape(-1, 128).T.astype(BF16).copy(),
            "dl_hi": dl_hi[c].reshape(-1, 128).T.astype(BF16).copy(),
        }
        for i in range(3):
            in_map[f"W{i+1}m"] = Ws[i].copy()
            in_map[f"b{i+1}m"] = np.broadcast_to(bs[i], (128, F)).copy()
        in_maps.append(in_map)

    if sim:
        from concourse.bass_interp import MultiCoreSim
        mcs = MultiCoreSim(nc, num_cores=N_CORES, trace=False,
                           require_finite=False, require_nnan=False)
        for ci, core in enumerate(mcs.cores.values()):
            for k, v in in_maps[ci].items():
                core.tensor(k)[:] = v
        mcs.simulate(check_with_hw=False)
        outs = [np.asarray(core.tensor("h_out"))
                for core in mcs.cores.values()]
        res = None
    else:
        res = bass_utils.run_bass_kernel_spmd(
            nc, in_maps, core_ids=list(range(N_CORES)), trace=trace)
        outs = [r["h_out"] for r in res.results]
    full = np.concatenate(outs, axis=0)[:N_NODES]
    return full, res


def kernel(**inputs) -> np.ndarray:
    edge_index = np.asarray(inputs["edge_index"])
    prep = _preprocess(edge_index)
    out, _ = _build_and_run(inputs, *prep)
    return out
